# revision 2
# baseline (speedup 1.0000x reference)
"""Trainium2 Bass kernel for nn_DecoderV1 (dilated-conv decoder, 24-step recurrence).

Strategy: pure data parallel over batch (2048 -> 8 cores x 256). Inside a core,
activations live channel-major ([ch, batch] on [partitions, free]) in bf16; the
24x6 (step x layer) recurrence is emitted wavefront-ordered (w = t + l) as
straight-line Tile code so independent blocks pipeline across engines.

Per block (l, t): one K=128 matmul computes W2.T@state + W3.T@x (rhs = [state;x]
stacked on partitions); tanh/sigmoid split f/g partition halves (sigmoid output
realigned to rows 0:64 via the PSUM cross-partition read); one K=64 matmul for
W4; residual/state updates are fused scalar_tensor_tensor adds reading res
straight from PSUM. Skips land in a 6-step SBUF ring consumed by accumulating
W5 matmuls; y = W6.T@relu(h)+b6 gathered into one output row.

Only the encoder tail (last d columns per dilation d, 63 of 168*6 positions) is
ever read, so the host slices/transposes it and feeds 2MB instead of 528MB.
"""
import numpy as np
import ml_dtypes

DIL = (1, 2, 4, 8, 16, 32)
CB = (0, 1, 3, 7, 15, 31)  # cumsum of DIL
NSLOT = 63
T = 24
B = 2048
NC = 8
BL = B // NC          # 256 batch per core
F = 64                # filters
NW = T + len(DIL) - 1  # 29 wavefronts

_CACHE = {}


def _build():
    import concourse.bass as bass
    import concourse.tile as tile
    import concourse.mybir as mybir

    F32, BF16 = mybir.dt.float32, mybir.dt.bfloat16
    AF = mybir.ActivationFunctionType
    OP = mybir.AluOpType

    nc = bass.Bass("TRN2", target_bir_lowering=False, debug=False)

    enc_d = nc.dram_tensor("enc_tails", [F, NSLOT * BL], BF16, kind="ExternalInput")
    xf_d = nc.dram_tensor("xfeat", [16, T * BL], BF16, kind="ExternalInput")
    w23_d = nc.dram_tensor("w23", [128, 128], BF16, kind="ExternalInput")
    w1_d = nc.dram_tensor("w1", [16, 64], BF16, kind="ExternalInput")
    w4_d = nc.dram_tensor("w4", [64, 128], BF16, kind="ExternalInput")
    w5_d = nc.dram_tensor("w5", [64, 6 * 128], BF16, kind="ExternalInput")
    w6_d = nc.dram_tensor("w6", [128, 1], BF16, kind="ExternalInput")
    b1_d = nc.dram_tensor("b1", [128, 1], F32, kind="ExternalInput")
    b2_d = nc.dram_tensor("b2", [128, 1], F32, kind="ExternalInput")
    b4_d = nc.dram_tensor("b4", [128, 1], F32, kind="ExternalInput")
    b5_d = nc.dram_tensor("b5", [128, 1], F32, kind="ExternalInput")
    b6_d = nc.dram_tensor("b6", [1, 1], F32, kind="ExternalInput")
    y_d = nc.dram_tensor("y", [1, T * BL], F32, kind="ExternalOutput")

    with tile.TileContext(nc) as tc:
        with tc.tile_pool(name="const", bufs=1) as cpool, \
             tc.tile_pool(name="work", bufs=1) as wpool, \
             tc.tile_pool(name="psum", bufs=1, space="PSUM") as ppool:

            circ = cpool.tile([F, NSLOT * BL], BF16)
            xfeat = cpool.tile([16, T * BL], BF16)
            w23 = cpool.tile([128, 128], BF16)
            w1 = cpool.tile([16, 64], BF16)
            w4 = cpool.tile([64, 128], BF16)
            w5 = cpool.tile([64, 6 * 128], BF16)
            w6 = cpool.tile([128, 1], BF16)
            b1 = cpool.tile([128, 1], F32)
            b2 = cpool.tile([128, 1], F32)
            b4 = cpool.tile([128, 1], F32)
            b5 = cpool.tile([128, 1], F32)
            b6 = cpool.tile([1, 1], F32)
            x0_all = cpool.tile([128, T * BL], BF16)   # rows 64:128 hold x0
            ring = cpool.tile([F, 6 * 6 * BL], BF16)   # skips ring, slab=(t%6)*1536
            y_all = cpool.tile([1, T * BL], F32)

            for tl, dr in [(circ, enc_d), (xfeat, xf_d), (w23, w23_d), (w1, w1_d),
                           (w4, w4_d), (w5, w5_d), (w6, w6_d), (b1, b1_d),
                           (b2, b2_d), (b4, b4_d), (b5, b5_d), (b6, b6_d)]:
                nc.sync.dma_start(tl[:], dr.ap())

            rhs_tiles = {}

            def get_rhs(w):
                if w not in rhs_tiles:
                    rhs_tiles[w] = wpool.tile([128, 6 * BL], BF16, tag="rhs", bufs=3, name=f"rhs{w}")
                return rhs_tiles[w]

            def x0_chunk(c):
                # steps 2c, 2c+1 -> x0_all[64:128, c*512:(c+1)*512]
                xp = ppool.tile([128, 512], F32, tag="aux", bufs=2)
                nc.tensor.matmul(xp[64:128, :], w1[:], xfeat[:, c * 512:(c + 1) * 512],
                                 start=True, stop=True, tile_position=(0, 64))
                nc.scalar.activation(x0_all[64:128, c * 512:(c + 1) * 512],
                                     xp[64:128, :], AF.Tanh, bias=b1[64:128, :])

            x0_chunk(0)

            for w in range(NW):
                lmin, lmax = max(0, w - (T - 1)), min(5, w)
                nblk = lmax - lmin + 1
                c0, c1 = lmin * BL, (lmax + 1) * BL
                rhs = get_rhs(w)

                if w % 2 == 0 and w // 2 + 1 < 12:
                    x0_chunk(w // 2 + 1)

                # (0) prologue: state copies + x0 feed (gpsimd, aligned rows)
                for l in range(lmax, lmin - 1, -1):
                    t = w - l
                    slot = CB[l] + (t % DIL[l])
                    nc.gpsimd.tensor_copy(rhs[0:F, l * BL:(l + 1) * BL],
                                          circ[:, slot * BL:(slot + 1) * BL])
                if w <= T - 1:
                    nc.gpsimd.tensor_copy(rhs[64:128, 0:BL],
                                          x0_all[64:128, w * BL:(w + 1) * BL])

                # (1) dc matmuls
                dc = ppool.tile([128, 6 * BL], F32, tag="dc", bufs=1)
                for l in range(lmax, lmin - 1, -1):
                    nc.tensor.matmul(dc[:, l * BL:(l + 1) * BL], w23[:],
                                     rhs[:, l * BL:(l + 1) * BL], start=True, stop=True)

                # (2) activations: f=tanh rows 0:64; g=sigmoid rows 64:128 -> realigned
                fsb = wpool.tile([F, 6 * BL], BF16, tag="fsb", bufs=2)
                gsb = wpool.tile([F, 6 * BL], BF16, tag="gsb", bufs=2)
                nc.scalar.activation(fsb[:, c0:c1], dc[0:64, c0:c1], AF.Tanh,
                                     bias=b2[0:64, :])
                nc.scalar.activation(gsb[:, c0:c1], dc[64:128, c0:c1], AF.Sigmoid,
                                     bias=b2[64:128, :])

                # (3) gated
                gated = wpool.tile([F, 6 * BL], BF16, tag="gated", bufs=2)
                nc.vector.tensor_tensor(out=gated[:, c0:c1], in0=fsb[:, c0:c1],
                                        in1=gsb[:, c0:c1], op=OP.mult)

                # (4) W4 matmuls
                outp = ppool.tile([128, 6 * BL], F32, tag="out", bufs=1)
                for l in range(lmax, lmin - 1, -1):
                    nc.tensor.matmul(outp[:, l * BL:(l + 1) * BL], w4[:],
                                     gated[:, l * BL:(l + 1) * BL], start=True, stop=True)

                # (5) feed adds: x_l = x_{l-1} + res_l + b4r -> rhs_{w+1} x-half, one op
                lf0, lf1 = lmin, min(4, lmax, (w - 0))
                # feed for l in [lf0..lf1] valid iff t=w-l <= 23 (true: l>=lmin>=w-23)
                if lf1 >= lf0 and w + 1 < NW:
                    nrhs = get_rhs(w + 1)
                    nc.vector.scalar_tensor_tensor(
                        out=nrhs[64:128, (lf0 + 1) * BL:(lf1 + 2) * BL],
                        in0=outp[64:128, lf0 * BL:(lf1 + 1) * BL],
                        scalar=b4[64:128, :],
                        in1=rhs[64:128, lf0 * BL:(lf1 + 1) * BL],
                        op0=OP.add, op1=OP.add)

                # (6) state adds into circ (same value as feed, written to rows 0:64)
                for l in range(min(4, lmax), lmin - 1, -1):
                    t = w - l
                    if t + DIL[l] <= T - 1:
                        slot = CB[l] + (t % DIL[l])
                        nc.vector.scalar_tensor_tensor(
                            out=circ[:, slot * BL:(slot + 1) * BL],
                            in0=outp[64:128, l * BL:(l + 1) * BL],
                            scalar=b4[64:128, :],
                            in1=rhs[64:128, l * BL:(l + 1) * BL],
                            op0=OP.add, op1=OP.add)

                # (7) skips relu -> ring
                for l in range(lmax, lmin - 1, -1):
                    t = w - l
                    dst = (t % 6) * (6 * BL) + l * BL
                    nc.vector.tensor_scalar(
                        out=ring[:, dst:dst + BL], in0=outp[0:64, l * BL:(l + 1) * BL],
                        scalar1=b4[0:64, :], scalar2=0.0, op0=OP.add, op1=OP.max)

                # (8) step completion: W5 -> h -> y for step s = w-5
                s = w - 5
                if s >= 0:
                    hp = ppool.tile([128, BL], F32, tag="aux", bufs=2)
                    base = (s % 6) * (6 * BL)
                    for l in range(6):
                        nc.tensor.matmul(hp[:], w5[:, l * 128:(l + 1) * 128],
                                         ring[:, base + l * BL:base + (l + 1) * BL],
                                         start=(l == 0), stop=(l == 5))
                    hsb = wpool.tile([128, BL], BF16, tag="hsb", bufs=3)
                    nc.scalar.activation(hsb[:], hp[:], AF.Relu, bias=b5[:])
                    yp = ppool.tile([1, BL], F32, tag="aux", bufs=2)
                    nc.tensor.matmul(yp[:], w6[:], hsb[:], start=True, stop=True)
                    nc.scalar.activation(y_all[:, s * BL:(s + 1) * BL], yp[:],
                                         AF.Identity, bias=b6[:])

            nc.sync.dma_start(y_d.ap(), y_all[:])

    _split_multi_waits(nc)
    return nc


def _split_multi_waits(nc, max_waits: int = 1) -> int:
    """This walrus build encodes at most one sync wait per instruction; hoist
    extras onto same-engine EventSemaphore wait-nops (as raw bass emits)."""
    import concourse.mybir as mybir
    n = 0
    for f in nc.m.functions:
        for bb in f.blocks:
            insts = bb.instructions
            if not any(i.sync_info and i.sync_info.on_wait
                       and len(i.sync_info.on_wait) > max_waits for i in insts):
                continue
            new = []
            for inst in insts:
                si = inst.sync_info
                if si is not None and si.on_wait and len(si.on_wait) > max_waits:
                    waits = list(si.on_wait)
                    for j, wt in enumerate(waits[:-max_waits]):
                        new.append(mybir.InstEventSemaphore(
                            name=f"{inst.name}_xw{j}", engine=inst.engine,
                            sync_info=mybir.SyncInfo(on_wait=[wt], on_update=[])))
                        n += 1
                    inst.sync_info = mybir.SyncInfo(
                        on_wait=waits[-max_waits:], on_update=list(si.on_update))
                new.append(inst)
            bb.instructions = new
    return n


def _prep_inputs(inputs):
    bf = ml_dtypes.bfloat16
    enc = np.asarray(inputs["encoder_outputs"], np.float32)   # [6,2048,168,64]
    df = np.asarray(inputs["decoder_features"], np.float32)   # [2048,24,15]
    di = np.asarray(inputs["decoder_init_input"], np.float32)  # [2048,1]

    W = {k: np.asarray(inputs[k], np.float32) for k in
         ["W1", "W2", "W3", "W4", "W5", "W6", "b1", "b2", "b4", "b5", "b6"]}
    w23 = np.concatenate([W["W2"], W["W3"]], axis=0).astype(bf)      # [128,128]
    w5 = np.concatenate([W["W5"][l * 64:(l + 1) * 64, :] for l in range(6)],
                        axis=1).astype(bf)                            # [64, 768]
    b1 = np.zeros((128, 1), np.float32); b1[64:128, 0] = W["b1"]
    b2 = W["b2"].reshape(128, 1).astype(np.float32)
    b4 = W["b4"].reshape(128, 1).astype(np.float32)
    b5 = W["b5"].reshape(128, 1).astype(np.float32)
    b6 = W["b6"].reshape(1, 1).astype(np.float32)

    # encoder tails: slot CB[l]+j holds enc[l][:, 168-d+j, :].T  -> [64, 63*BL]
    in_maps = []
    for c in range(NC):
        bs = slice(c * BL, (c + 1) * BL)
        et = np.empty((F, NSLOT * BL), np.float32)
        for l, d in enumerate(DIL):
            # [d, BL, F] -> [F, d, BL]
            blk = enc[l, bs, 168 - d:168, :].transpose(2, 0, 1)[:, :, :]
            blk = np.ascontiguousarray(np.transpose(enc[l, bs, 168 - d:168, :], (2, 1, 0)))
            et[:, CB[l] * BL:(CB[l] + d) * BL] = blk.reshape(F, d * BL)
        xf = np.empty((16, T * BL), np.float32)
        xf[0] = np.repeat(di[bs, 0][None, :], T, axis=0).reshape(T * BL)
        xf[1:16] = df[bs].transpose(2, 1, 0).reshape(15, T * BL)  # [15, T, BL]
        in_maps.append({
            "enc_tails": et.astype(bf), "xfeat": xf.astype(bf),
            "w23": w23, "w1": W["W1"].astype(bf), "w4": W["W4"].astype(bf),
            "w5": w5, "w6": W["W6"].astype(bf),
            "b1": b1, "b2": b2, "b4": b4, "b5": b5, "b6": b6,
        })
    return in_maps


def kernel(**inputs) -> np.ndarray:
    from concourse.bass_utils import run_bass_kernel_spmd
    if "nc" not in _CACHE:
        _CACHE["nc"] = _build()
    nc = _CACHE["nc"]
    in_maps = _prep_inputs(inputs)
    res = run_bass_kernel_spmd(nc, in_maps, core_ids=list(range(NC)))
    out = np.empty((B, T, 1), np.float32)
    for c in range(NC):
        y = res.results[c]["y"].reshape(T, BL)  # [t, b]
        out[c * BL:(c + 1) * BL, :, 0] = y.T
    return out


# revision 3
# speedup vs baseline: 1.4026x; 1.4026x over previous
"""Trainium2 Bass kernel for nn_DecoderV1 (dilated-conv decoder, 24-step recurrence).

Strategy: pure data parallel over batch (2048 -> 8 cores x 256). Inside a core,
activations live channel-major ([ch, batch] on [partitions, free]) in bf16; the
24x6 (step x layer) recurrence is emitted wavefront-ordered (w = t + l) as
straight-line Tile code so independent blocks pipeline across engines.

Per block (l, t): one K=128 matmul computes W2.T@state + W3.T@x (rhs = [state;x]
stacked on partitions); tanh/sigmoid split f/g partition halves (sigmoid output
realigned to rows 0:64 via the PSUM cross-partition read); one K=64 matmul for
W4; residual/state updates are fused scalar_tensor_tensor adds reading res
straight from PSUM. Skips land in a 6-step SBUF ring consumed by accumulating
W5 matmuls; y = W6.T@relu(h)+b6 gathered into one output row.

Only the encoder tail (last d columns per dilation d, 63 of 168*6 positions) is
ever read, so the host slices/transposes it and feeds 2MB instead of 528MB.
"""
import numpy as np
import ml_dtypes

DIL = (1, 2, 4, 8, 16, 32)
CB = (0, 1, 3, 7, 15, 31)  # cumsum of DIL
NSLOT = 63
T = 24
B = 2048
NC = 8
BL = B // NC          # 256 batch per core
F = 64                # filters
NW = T + len(DIL) - 1  # 29 wavefronts

_CACHE = {}


def _build():
    import concourse.bass as bass
    import concourse.tile as tile
    import concourse.mybir as mybir

    F32, BF16 = mybir.dt.float32, mybir.dt.bfloat16
    AF = mybir.ActivationFunctionType
    OP = mybir.AluOpType

    nc = bass.Bass("TRN2", target_bir_lowering=False, debug=False)

    enc_d = nc.dram_tensor("enc_tails", [F, NSLOT * BL], BF16, kind="ExternalInput")
    xf_d = nc.dram_tensor("xfeat", [16, T * BL], BF16, kind="ExternalInput")
    w23_d = nc.dram_tensor("w23", [128, 128], BF16, kind="ExternalInput")
    w1_d = nc.dram_tensor("w1", [16, 64], BF16, kind="ExternalInput")
    w4_d = nc.dram_tensor("w4", [64, 128], BF16, kind="ExternalInput")
    w5_d = nc.dram_tensor("w5", [64, 6 * 128], BF16, kind="ExternalInput")
    w6_d = nc.dram_tensor("w6", [128, 1], BF16, kind="ExternalInput")
    b1_d = nc.dram_tensor("b1", [128, 1], F32, kind="ExternalInput")
    b2_d = nc.dram_tensor("b2", [128, 1], F32, kind="ExternalInput")
    b4_d = nc.dram_tensor("b4", [128, 1], F32, kind="ExternalInput")
    b5_d = nc.dram_tensor("b5", [128, 1], F32, kind="ExternalInput")
    b6_d = nc.dram_tensor("b6", [1, 1], F32, kind="ExternalInput")
    y_d = nc.dram_tensor("y", [1, T * BL], F32, kind="ExternalOutput")

    with tile.TileContext(nc) as tc:
        with tc.tile_pool(name="const", bufs=1) as cpool, \
             tc.tile_pool(name="work", bufs=1) as wpool, \
             tc.tile_pool(name="psum", bufs=1, space="PSUM") as ppool:

            circ = cpool.tile([F, NSLOT * BL], BF16)
            xfeat = cpool.tile([16, T * BL], BF16)
            w23 = cpool.tile([128, 128], BF16)
            w1 = cpool.tile([16, 64], BF16)
            w4 = cpool.tile([64, 128], BF16)
            w5 = cpool.tile([64, 6 * 128], BF16)
            w6 = cpool.tile([128, 1], BF16)
            b1 = cpool.tile([128, 1], F32)
            b2 = cpool.tile([128, 1], F32)
            b4 = cpool.tile([128, 1], F32)
            b5 = cpool.tile([128, 1], F32)
            b6 = cpool.tile([1, 1], F32)
            x0_all = cpool.tile([128, T * BL], BF16)   # rows 64:128 hold x0
            ring = cpool.tile([F, 6 * 6 * BL], BF16)   # skips ring, slab=(t%6)*1536
            y_all = cpool.tile([1, T * BL], F32)

            for tl, dr in [(circ, enc_d), (xfeat, xf_d), (w23, w23_d), (w1, w1_d),
                           (w4, w4_d), (w5, w5_d), (w6, w6_d), (b1, b1_d),
                           (b2, b2_d), (b4, b4_d), (b5, b5_d), (b6, b6_d)]:
                nc.sync.dma_start(tl[:], dr.ap())

            rhs_tiles = {}

            def get_rhs(w):
                if w not in rhs_tiles:
                    rhs_tiles[w] = wpool.tile([128, 6 * BL], BF16, tag="rhs", bufs=17, name=f"rhs{w}")
                return rhs_tiles[w]

            def x0_chunk(c):
                # steps 2c, 2c+1 -> x0_all[64:128, c*512:(c+1)*512]
                xp = ppool.tile([128, 512], F32, tag="aux", bufs=2)
                nc.tensor.matmul(xp[64:128, :], w1[:], xfeat[:, c * 512:(c + 1) * 512],
                                 start=True, stop=True, tile_position=(0, 64))
                nc.scalar.activation(x0_all[64:128, c * 512:(c + 1) * 512],
                                     xp[64:128, :], AF.Tanh, bias=b1[64:128, :])

            x0_chunk(0)

            for w in range(NW):
                lmin, lmax = max(0, w - (T - 1)), min(5, w)
                nblk = lmax - lmin + 1
                c0, c1 = lmin * BL, (lmax + 1) * BL
                rhs = get_rhs(w)

                if w % 2 == 0 and w // 2 + 1 < 12:
                    x0_chunk(w // 2 + 1)

                # (0) prologue: state copies + x0 feed (gpsimd, aligned rows)
                for l in range(lmax, lmin - 1, -1):
                    t = w - l
                    if t < DIL[l]:
                        slot = CB[l] + t
                        nc.gpsimd.tensor_copy(rhs[0:F, l * BL:(l + 1) * BL],
                                              circ[:, slot * BL:(slot + 1) * BL])
                    else:
                        src_t = rhs_tiles[w - DIL[l] + 1]
                        nc.gpsimd.tensor_copy(rhs[0:F, l * BL:(l + 1) * BL],
                                              src_t[64:128, (l + 1) * BL:(l + 2) * BL])
                if w <= T - 1:
                    nc.gpsimd.tensor_copy(rhs[64:128, 0:BL],
                                          x0_all[64:128, w * BL:(w + 1) * BL])

                # (1) dc matmuls
                dc = ppool.tile([128, 6 * BL], F32, tag="dc", bufs=1)
                for l in range(lmax, lmin - 1, -1):
                    nc.tensor.matmul(dc[:, l * BL:(l + 1) * BL], w23[:],
                                     rhs[:, l * BL:(l + 1) * BL], start=True, stop=True)

                # (2) one full-width tanh; g-half holds tanh(g/2) (0.5 folded
                # into w23/b2 host-side); sigma = 0.5*th_g+0.5 realigned to rows 0:64
                th = wpool.tile([128, 6 * BL], BF16, tag="th", bufs=2)
                nc.scalar.activation(th[:, c0:c1], dc[:, c0:c1], AF.Tanh, bias=b2[:])
                ssb = wpool.tile([F, 6 * BL], BF16, tag="ssb", bufs=2)
                nc.vector.tensor_scalar(out=ssb[:, c0:c1], in0=th[64:128, c0:c1],
                                        scalar1=0.5, scalar2=0.5,
                                        op0=OP.mult, op1=OP.add)

                # (3) gated = tanh(f) * sigmoid(g)
                gated = wpool.tile([F, 6 * BL], BF16, tag="gated", bufs=2)
                nc.vector.tensor_tensor(out=gated[:, c0:c1], in0=th[0:64, c0:c1],
                                        in1=ssb[:, c0:c1], op=OP.mult)

                # (4) W4 matmuls
                outp = ppool.tile([128, 6 * BL], F32, tag="out", bufs=1)
                for l in range(lmax, lmin - 1, -1):
                    nc.tensor.matmul(outp[:, l * BL:(l + 1) * BL], w4[:],
                                     gated[:, l * BL:(l + 1) * BL], start=True, stop=True)

                # (5) feed adds: x_l = x_{l-1} + res_l + b4r -> rhs_{w+1} x-half, one op
                lf0, lf1 = lmin, min(4, lmax, (w - 0))
                # feed for l in [lf0..lf1] valid iff t=w-l <= 23 (true: l>=lmin>=w-23)
                if lf1 >= lf0 and w + 1 < NW:
                    nrhs = get_rhs(w + 1)
                    nc.vector.scalar_tensor_tensor(
                        out=nrhs[64:128, (lf0 + 1) * BL:(lf1 + 2) * BL],
                        in0=outp[64:128, lf0 * BL:(lf1 + 1) * BL],
                        scalar=b4[64:128, :],
                        in1=rhs[64:128, lf0 * BL:(lf1 + 1) * BL],
                        op0=OP.add, op1=OP.add)

                # (7) skips relu -> ring slab (w % 6), contiguous batch
                rbase = (w % 6) * (6 * BL)
                if w % 2 == 0:
                    nc.vector.tensor_scalar(
                        out=ring[:, rbase + c0:rbase + c1], in0=outp[0:64, c0:c1],
                        scalar1=b4[0:64, :], scalar2=0.0, op0=OP.add, op1=OP.max)
                else:
                    nc.scalar.activation(ring[:, rbase + c0:rbase + c1],
                                         outp[0:64, c0:c1], AF.Relu, bias=b4[0:64, :])

                # (8) step completion: W5 -> h -> y for step s = w-5
                s = w - 5
                if s >= 0:
                    hp = ppool.tile([128, BL], F32, tag="aux", bufs=2)
                    for l in range(6):
                        base = ((s + l) % 6) * (6 * BL)
                        nc.tensor.matmul(hp[:], w5[:, l * 128:(l + 1) * 128],
                                         ring[:, base + l * BL:base + (l + 1) * BL],
                                         start=(l == 0), stop=(l == 5))
                    hsb = wpool.tile([128, BL], BF16, tag="hsb", bufs=3)
                    nc.scalar.activation(hsb[:], hp[:], AF.Relu, bias=b5[:])
                    yp = ppool.tile([1, BL], F32, tag="aux", bufs=2)
                    nc.tensor.matmul(yp[:], w6[:], hsb[:], start=True, stop=True)
                    nc.scalar.activation(y_all[:, s * BL:(s + 1) * BL], yp[:],
                                         AF.Identity, bias=b6[:])

            nc.sync.dma_start(y_d.ap(), y_all[:])

    _split_multi_waits(nc)
    return nc


def _split_multi_waits(nc, max_waits: int = 1) -> int:
    """This walrus build encodes at most one sync wait per instruction; hoist
    extras onto same-engine EventSemaphore wait-nops (as raw bass emits)."""
    import concourse.mybir as mybir
    n = 0
    for f in nc.m.functions:
        for bb in f.blocks:
            insts = bb.instructions
            if not any(i.sync_info and i.sync_info.on_wait
                       and len(i.sync_info.on_wait) > max_waits for i in insts):
                continue
            new = []
            for inst in insts:
                si = inst.sync_info
                if si is not None and si.on_wait and len(si.on_wait) > max_waits:
                    waits = list(si.on_wait)
                    for j, wt in enumerate(waits[:-max_waits]):
                        new.append(mybir.InstEventSemaphore(
                            name=f"{inst.name}_xw{j}", engine=inst.engine,
                            sync_info=mybir.SyncInfo(on_wait=[wt], on_update=[])))
                        n += 1
                    inst.sync_info = mybir.SyncInfo(
                        on_wait=waits[-max_waits:], on_update=list(si.on_update))
                new.append(inst)
            bb.instructions = new
    return n


def _prep_inputs(inputs):
    bf = ml_dtypes.bfloat16
    enc = np.asarray(inputs["encoder_outputs"], np.float32)   # [6,2048,168,64]
    df = np.asarray(inputs["decoder_features"], np.float32)   # [2048,24,15]
    di = np.asarray(inputs["decoder_init_input"], np.float32)  # [2048,1]

    W = {k: np.asarray(inputs[k], np.float32) for k in
         ["W1", "W2", "W3", "W4", "W5", "W6", "b1", "b2", "b4", "b5", "b6"]}
    w23f = np.concatenate([W["W2"], W["W3"]], axis=0)                 # [128,128]
    w23f[:, 64:128] *= 0.5       # sigmoid(g) = 0.5*tanh(g/2)+0.5
    w23 = w23f.astype(bf)
    w5 = np.concatenate([W["W5"][l * 64:(l + 1) * 64, :] for l in range(6)],
                        axis=1).astype(bf)                            # [64, 768]
    b1 = np.zeros((128, 1), np.float32); b1[64:128, 0] = W["b1"]
    b2 = W["b2"].reshape(128, 1).astype(np.float32).copy()
    b2[64:128] *= 0.5
    b4 = W["b4"].reshape(128, 1).astype(np.float32)
    b5 = W["b5"].reshape(128, 1).astype(np.float32)
    b6 = W["b6"].reshape(1, 1).astype(np.float32)

    # encoder tails: slot CB[l]+j holds enc[l][:, 168-d+j, :].T  -> [64, 63*BL]
    in_maps = []
    for c in range(NC):
        bs = slice(c * BL, (c + 1) * BL)
        et = np.empty((F, NSLOT * BL), np.float32)
        for l, d in enumerate(DIL):
            # [d, BL, F] -> [F, d, BL]
            blk = enc[l, bs, 168 - d:168, :].transpose(2, 0, 1)[:, :, :]
            blk = np.ascontiguousarray(np.transpose(enc[l, bs, 168 - d:168, :], (2, 1, 0)))
            et[:, CB[l] * BL:(CB[l] + d) * BL] = blk.reshape(F, d * BL)
        xf = np.empty((16, T * BL), np.float32)
        xf[0] = np.repeat(di[bs, 0][None, :], T, axis=0).reshape(T * BL)
        xf[1:16] = df[bs].transpose(2, 1, 0).reshape(15, T * BL)  # [15, T, BL]
        in_maps.append({
            "enc_tails": et.astype(bf), "xfeat": xf.astype(bf),
            "w23": w23, "w1": W["W1"].astype(bf), "w4": W["W4"].astype(bf),
            "w5": w5, "w6": W["W6"].astype(bf),
            "b1": b1, "b2": b2, "b4": b4, "b5": b5, "b6": b6,
        })
    return in_maps


def kernel(**inputs) -> np.ndarray:
    from concourse.bass_utils import run_bass_kernel_spmd
    if "nc" not in _CACHE:
        _CACHE["nc"] = _build()
    nc = _CACHE["nc"]
    in_maps = _prep_inputs(inputs)
    res = run_bass_kernel_spmd(nc, in_maps, core_ids=list(range(NC)))
    out = np.empty((B, T, 1), np.float32)
    for c in range(NC):
        y = res.results[c]["y"].reshape(T, BL)  # [t, b]
        out[c * BL:(c + 1) * BL, :, 0] = y.T
    return out


# revision 5
# speedup vs baseline: 1.4914x; 1.0634x over previous
"""Trainium2 Bass kernel for nn_DecoderV1 (dilated-conv decoder, 24-step recurrence).

Strategy: pure data parallel over batch (2048 -> 8 cores x 256). Inside a core,
activations live channel-major ([ch, batch] on [partitions, free]) in bf16; the
24x6 (step x layer) recurrence is emitted wavefront-ordered (w = t + l) as
straight-line Tile code so independent blocks pipeline across engines.

Per block (l, t): one K=128 matmul computes W2.T@state + W3.T@x (rhs = [state;x]
stacked on partitions); tanh/sigmoid split f/g partition halves (sigmoid output
realigned to rows 0:64 via the PSUM cross-partition read); one K=64 matmul for
W4; residual/state updates are fused scalar_tensor_tensor adds reading res
straight from PSUM. Skips land in a 6-step SBUF ring consumed by accumulating
W5 matmuls; y = W6.T@relu(h)+b6 gathered into one output row.

Only the encoder tail (last d columns per dilation d, 63 of 168*6 positions) is
ever read, so the host slices/transposes it and feeds 2MB instead of 528MB.
"""
import numpy as np
import ml_dtypes

DIL = (1, 2, 4, 8, 16, 32)
CB = (0, 1, 3, 7, 15, 31)  # cumsum of DIL
NSLOT = 63
T = 24
B = 2048
NC = 8
BL = B // NC          # 256 batch per core
F = 64                # filters
NW = T + len(DIL) - 1  # 29 wavefronts

_CACHE = {}


def _build():
    import concourse.bass as bass
    import concourse.tile as tile
    import concourse.mybir as mybir

    F32, BF16 = mybir.dt.float32, mybir.dt.bfloat16
    AF = mybir.ActivationFunctionType
    OP = mybir.AluOpType

    nc = bass.Bass("TRN2", target_bir_lowering=False, debug=False)

    enc_ds = [nc.dram_tensor(f"enc{l}", [F, DIL[l] * BL], BF16, kind="ExternalInput")
              for l in range(6)]
    xf_d = nc.dram_tensor("xfeat", [16, T * BL], BF16, kind="ExternalInput")
    w2_d = nc.dram_tensor("w2", [64, 128], BF16, kind="ExternalInput")
    w3_d = nc.dram_tensor("w3", [64, 128], BF16, kind="ExternalInput")
    w1_d = nc.dram_tensor("w1", [16, 64], BF16, kind="ExternalInput")
    w4_d = nc.dram_tensor("w4", [64, 128], BF16, kind="ExternalInput")
    w5_d = nc.dram_tensor("w5", [64, 6 * 128], BF16, kind="ExternalInput")
    w6_d = nc.dram_tensor("w6", [128, 1], BF16, kind="ExternalInput")
    b1_d = nc.dram_tensor("b1", [128, 1], F32, kind="ExternalInput")
    b2_d = nc.dram_tensor("b2", [128, 1], F32, kind="ExternalInput")
    b4_d = nc.dram_tensor("b4", [128, 1], F32, kind="ExternalInput")
    b4r_d = nc.dram_tensor("b4r", [64, 1], F32, kind="ExternalInput")
    b5_d = nc.dram_tensor("b5", [128, 1], F32, kind="ExternalInput")
    b6_d = nc.dram_tensor("b6", [1, 1], F32, kind="ExternalInput")
    y_d = nc.dram_tensor("y", [1, T * BL], F32, kind="ExternalOutput")

    with tile.TileContext(nc) as tc:
        with tc.tile_pool(name="const", bufs=1) as cpool, \
             tc.tile_pool(name="work", bufs=1) as wpool, \
             tc.tile_pool(name="psum", bufs=1, space="PSUM") as ppool:

            circs = [cpool.tile([F, DIL[l] * BL], BF16, name=f"circ{l}")
                     for l in range(6)]
            xfeat = cpool.tile([16, T * BL], BF16)
            w2 = cpool.tile([64, 128], BF16)
            w3 = cpool.tile([64, 128], BF16)
            w1 = cpool.tile([16, 64], BF16)
            w4 = cpool.tile([64, 128], BF16)
            w5 = cpool.tile([64, 6 * 128], BF16)
            w6 = cpool.tile([128, 1], BF16)
            b1 = cpool.tile([128, 1], F32)
            b2 = cpool.tile([128, 1], F32)
            b4 = cpool.tile([128, 1], F32)
            b4r = cpool.tile([64, 1], F32)
            b5 = cpool.tile([128, 1], F32)
            b6 = cpool.tile([1, 1], F32)
            x0_all = cpool.tile([F, T * BL], BF16)
            ring = cpool.tile([F, 6 * 6 * BL], BF16)   # slab = (w%6)*1536
            y_all = cpool.tile([1, T * BL], F32)

            for tl, dr in ([(xfeat, xf_d), (w2, w2_d), (w3, w3_d), (w1, w1_d),
                            (w4, w4_d), (w5, w5_d), (w6, w6_d), (b1, b1_d),
                            (b2, b2_d), (b4, b4_d), (b4r, b4r_d), (b5, b5_d), (b6, b6_d)]
                           + [(circs[l], enc_ds[l]) for l in range(6)]):
                nc.sync.dma_start(tl[:], dr.ap())

            # x-history: xh[w][l*BL:(l+1)*BL] holds x_l for step t=w-l-1
            xh_tiles = {}

            def get_xh(w):
                if w not in xh_tiles:
                    xh_tiles[w] = wpool.tile([F, 5 * BL], BF16, tag="xh", bufs=17,
                                             name=f"xh{w}")
                return xh_tiles[w]

            def state_src(l, t):
                if t < DIL[l]:
                    return circs[l][:, t * BL:(t + 2 - 1) * BL]
                wsrc = (t - DIL[l]) + l + 1
                return xh_tiles[wsrc][:, l * BL:(l + 1) * BL]

            def x_src(l, t, w):
                # x_{l-1}^t
                if l == 0:
                    return x0_all[:, t * BL:(t + 1) * BL]
                return get_xh(w)[:, (l - 1) * BL:l * BL]

            def x0_chunk(c):
                xp = ppool.tile([64, 512], F32, tag="aux", bufs=2, name=f"x0p{c}")
                nc.tensor.matmul(xp[:], w1[:], xfeat[:, c * 512:(c + 1) * 512],
                                 start=True, stop=True)
                nc.scalar.activation(x0_all[:, c * 512:(c + 1) * 512],
                                     xp[:], AF.Tanh, bias=b1[0:64, :])

            x0_chunk(0)

            for w in range(NW):
                lmin, lmax = max(0, w - (T - 1)), min(5, w)
                c0, c1 = lmin * BL, (lmax + 1) * BL
                has0 = lmin == 0

                if w % 2 == 0 and w // 2 + 1 < 12:
                    x0_chunk(w // 2 + 1)

                dc = ppool.tile([128, 6 * BL], F32, tag="dc", bufs=1, name=f"dc{w}")
                outp = ppool.tile([128, 6 * BL], F32, tag="out", bufs=1, name=f"o{w}")
                th = wpool.tile([128, 6 * BL], BF16, tag="th", bufs=2, name=f"th{w}")
                ssb = wpool.tile([F, 6 * BL], BF16, tag="ssb", bufs=2, name=f"ss{w}")
                gated = wpool.tile([F, 6 * BL], BF16, tag="gated", bufs=2,
                                   name=f"gt{w}")
                rbase = (w % 6) * (6 * BL)

                def block_chain(l):
                    t = w - l
                    sl = l * BL
                    nc.tensor.matmul(dc[:, sl:sl + BL], w2[:], state_src(l, t),
                                     start=True, stop=False)
                    nc.tensor.matmul(dc[:, sl:sl + BL], w3[:], x_src(l, t, w),
                                     start=False, stop=True)

                # fast path: block 0 (the d=1 recurrence chain)
                if has0:
                    block_chain(0)
                    nc.scalar.activation(th[:, 0:BL], dc[:, 0:BL], AF.Tanh,
                                         bias=b2[:])
                    nc.vector.tensor_scalar(out=ssb[:, 0:BL], in0=th[64:128, 0:BL],
                                            scalar1=0.5, scalar2=0.5,
                                            op0=OP.mult, op1=OP.add)
                    nc.vector.tensor_tensor(out=gated[:, 0:BL], in0=th[0:64, 0:BL],
                                            in1=ssb[:, 0:BL], op=OP.mult)
                    nc.tensor.matmul(outp[:, 0:BL], w4[:], gated[:, 0:BL],
                                     start=True, stop=True)
                    if w + 1 < NW and w <= T - 1:
                        nxh = get_xh(w + 1)
                        nc.vector.scalar_tensor_tensor(
                            out=nxh[:, 0:BL], in0=outp[64:128, 0:BL],
                            scalar=b4r[:], in1=x0_all[:, w * BL:(w + 1) * BL],
                            op0=OP.add, op1=OP.add)

                # rest blocks
                r0 = max(1, lmin)
                if lmax >= r0:
                    rc0, rc1 = r0 * BL, (lmax + 1) * BL
                    for l in range(lmax, r0 - 1, -1):
                        block_chain(l)
                    nc.scalar.activation(th[:, rc0:rc1], dc[:, rc0:rc1], AF.Tanh,
                                         bias=b2[:])
                    nc.vector.tensor_scalar(out=ssb[:, rc0:rc1],
                                            in0=th[64:128, rc0:rc1],
                                            scalar1=0.5, scalar2=0.5,
                                            op0=OP.mult, op1=OP.add)
                    nc.vector.tensor_tensor(out=gated[:, rc0:rc1],
                                            in0=th[0:64, rc0:rc1],
                                            in1=ssb[:, rc0:rc1], op=OP.mult)
                    for l in range(lmax, r0 - 1, -1):
                        nc.tensor.matmul(outp[:, l * BL:(l + 1) * BL], w4[:],
                                         gated[:, l * BL:(l + 1) * BL],
                                         start=True, stop=True)
                    lf1 = min(4, lmax)
                    if lf1 >= r0 and w + 1 < NW:
                        nxh = get_xh(w + 1)
                        nc.vector.scalar_tensor_tensor(
                            out=nxh[:, r0 * BL:(lf1 + 1) * BL],
                            in0=outp[64:128, r0 * BL:(lf1 + 1) * BL],
                            scalar=b4r[:],
                            in1=get_xh(w)[:, (r0 - 1) * BL:lf1 * BL],
                            op0=OP.add, op1=OP.add)

                # skips relu -> ring slab (w%6), whole active range, alt engines
                if w % 2 == 0:
                    nc.vector.tensor_scalar(
                        out=ring[:, rbase + c0:rbase + c1], in0=outp[0:64, c0:c1],
                        scalar1=b4[0:64, :], scalar2=0.0, op0=OP.add, op1=OP.max)
                else:
                    nc.scalar.activation(ring[:, rbase + c0:rbase + c1],
                                         outp[0:64, c0:c1], AF.Relu, bias=b4[0:64, :])

                # step completion
                s = w - 5
                if s >= 0:
                    hp = ppool.tile([128, BL], F32, tag="aux", bufs=2, name=f"hp{s}")
                    for l in range(6):
                        base = ((s + l) % 6) * (6 * BL)
                        nc.tensor.matmul(hp[:], w5[:, l * 128:(l + 1) * 128],
                                         ring[:, base + l * BL:base + (l + 1) * BL],
                                         start=(l == 0), stop=(l == 5))
                    hsb = wpool.tile([128, BL], BF16, tag="hsb", bufs=3, name=f"hs{s}")
                    nc.scalar.activation(hsb[:], hp[:], AF.Relu, bias=b5[:])
                    yp = ppool.tile([1, BL], F32, tag="aux", bufs=2, name=f"yp{s}")
                    nc.tensor.matmul(yp[:], w6[:], hsb[:], start=True, stop=True)
                    nc.scalar.activation(y_all[:, s * BL:(s + 1) * BL], yp[:],
                                         AF.Identity, bias=b6[:])

            nc.sync.dma_start(y_d.ap(), y_all[:])

    _split_multi_waits(nc)
    return nc


def _split_multi_waits(nc, max_waits: int = 1) -> int:
    """This walrus build encodes at most one sync wait per instruction; hoist
    extras onto same-engine EventSemaphore wait-nops (as raw bass emits)."""
    import concourse.mybir as mybir
    n = 0
    for f in nc.m.functions:
        for bb in f.blocks:
            insts = bb.instructions
            if not any(i.sync_info and i.sync_info.on_wait
                       and len(i.sync_info.on_wait) > max_waits for i in insts):
                continue
            new = []
            for inst in insts:
                si = inst.sync_info
                if si is not None and si.on_wait and len(si.on_wait) > max_waits:
                    waits = list(si.on_wait)
                    for j, wt in enumerate(waits[:-max_waits]):
                        new.append(mybir.InstEventSemaphore(
                            name=f"{inst.name}_xw{j}", engine=inst.engine,
                            sync_info=mybir.SyncInfo(on_wait=[wt], on_update=[])))
                        n += 1
                    inst.sync_info = mybir.SyncInfo(
                        on_wait=waits[-max_waits:], on_update=list(si.on_update))
                new.append(inst)
            bb.instructions = new
    return n


def _prep_inputs(inputs):
    bf = ml_dtypes.bfloat16
    enc = np.asarray(inputs["encoder_outputs"], np.float32)   # [6,2048,168,64]
    df = np.asarray(inputs["decoder_features"], np.float32)   # [2048,24,15]
    di = np.asarray(inputs["decoder_init_input"], np.float32)  # [2048,1]

    W = {k: np.asarray(inputs[k], np.float32) for k in
         ["W1", "W2", "W3", "W4", "W5", "W6", "b1", "b2", "b4", "b5", "b6"]}
    w2 = W["W2"].copy(); w2[:, 64:128] *= 0.5   # sigmoid(g)=0.5*tanh(g/2)+0.5
    w3 = W["W3"].copy(); w3[:, 64:128] *= 0.5
    w5 = np.concatenate([W["W5"][l * 64:(l + 1) * 64, :] for l in range(6)],
                        axis=1).astype(bf)                            # [64, 768]
    b1 = np.zeros((128, 1), np.float32); b1[0:64, 0] = W["b1"]
    b2 = W["b2"].reshape(128, 1).astype(np.float32).copy(); b2[64:128] *= 0.5
    b4 = W["b4"].reshape(128, 1).astype(np.float32)
    b5 = W["b5"].reshape(128, 1).astype(np.float32)
    b6 = W["b6"].reshape(1, 1).astype(np.float32)

    in_maps = []
    for c in range(NC):
        bs = slice(c * BL, (c + 1) * BL)
        m = {
            "xfeat": None,
            "w2": w2.astype(bf), "w3": w3.astype(bf), "w1": W["W1"].astype(bf),
            "w4": W["W4"].astype(bf), "w5": w5, "w6": W["W6"].astype(bf),
            "b1": b1, "b2": b2, "b4": b4, "b4r": np.ascontiguousarray(b4[64:128]),
            "b5": b5, "b6": b6,
        }
        for l, d in enumerate(DIL):
            blk = np.ascontiguousarray(
                np.transpose(enc[l, bs, 168 - d:168, :], (2, 1, 0)))  # [F, d, BL]
            m[f"enc{l}"] = blk.reshape(F, d * BL).astype(bf)
        xf = np.empty((16, T * BL), np.float32)
        xf[0] = np.repeat(di[bs, 0][None, :], T, axis=0).reshape(T * BL)
        xf[1:16] = df[bs].transpose(2, 1, 0).reshape(15, T * BL)
        m["xfeat"] = xf.astype(bf)
        in_maps.append(m)
    return in_maps


def kernel(**inputs) -> np.ndarray:
    from concourse.bass_utils import run_bass_kernel_spmd
    if "nc" not in _CACHE:
        _CACHE["nc"] = _build()
    nc = _CACHE["nc"]
    in_maps = _prep_inputs(inputs)
    res = run_bass_kernel_spmd(nc, in_maps, core_ids=list(range(NC)))
    out = np.empty((B, T, 1), np.float32)
    for c in range(NC):
        y = res.results[c]["y"].reshape(T, BL)  # [t, b]
        out[c * BL:(c + 1) * BL, :, 0] = y.T
    return out
